# revision 1
# baseline (speedup 1.0000x reference)
"""Distributed embedding lookup v6: segment gathers + prescale-to-bf16 +
bf16 dma_scatter_add with a two-group staging pass for duplicate-row keys.

Sharding: output rows split contiguously over 8 cores (53,248 rows each).
Keys are routed host-side into per-core buckets keyed by vocab segment
(32,768 rows, int16 gather indices).  The HW CCE scatter-add loses
updates when one instruction carries two descriptors targeting the same
output row, so every scatter instruction must be row-unique:

- level-0 keys (rank 0 within (core, seg, half, row), ~97%) scatter
  directly per (segment, half) -- row-unique by construction; Tile's
  WAW chain serializes across instructions.
- tail keys (rank >= 1) are prescaled to f32 and scattered into a
  compact staging buffer at per-key unique slots (collision-free), then
  re-gathered in (half, global-level, row) order and added to the
  output by bundle scatters (one per (half, global tail level), each
  row-unique).  The tail pass runs per segment-GROUP (two groups) so
  the first group's re-gather and bundle scatters overlap the second
  group's main sweep instead of serializing at the end.

The mean combine is a per-key 1/count pre-scale fused with the dtype
convert, so output scatters move 128B bf16 descriptors into a
256B-stride bf16 output at half the f32 DMA cost.  Pad slots use gather
index 0 with recip=0 and scatter into per-half dummy rows (or their own
unique staging slot), keeping every index count compile-time static so
one NEFF serves all 8 cores.
"""

import numpy as np

CORES = 8
SEG = 32768  # vocab rows per gather segment (int16 index range)
HALF_PAD = 128  # dummy scatter rows appended per output half
NGRP = 1  # tail-pass segment groups


def _build_program(V, D, HR, plan):
    import concourse.bacc as bacc
    import concourse.mybir as mybir
    import concourse.tile as tile

    dt = mybir.dt
    TOT = plan["TOT"]
    STG = plan["STG"]
    TOTB = plan["TOTB"]
    MAXC = plan["MAXC"]
    MAXT = plan["MAXT"]
    MAXB = plan["MAXB"]
    HOUT = HR + HALF_PAD

    nc = bacc.Bacc("TRN2", target_bir_lowering=False, debug=False)
    table = nc.dram_tensor("table", [V, D], dt.float32, kind="ExternalInput").ap()
    gidx = nc.dram_tensor("gidx", [128, TOT // 16], dt.int16, kind="ExternalInput").ap()
    sidx = nc.dram_tensor("sidx", [128, TOT // 16], dt.int16, kind="ExternalInput").ap()
    recip = nc.dram_tensor("recip", [128, TOT // 128], dt.float32, kind="ExternalInput").ap()
    gidx2 = nc.dram_tensor("gidx2", [128, TOTB // 16], dt.int16, kind="ExternalInput").ap()
    sidx2 = nc.dram_tensor("sidx2", [128, TOTB // 16], dt.int16, kind="ExternalInput").ap()
    # bf16 outputs with 256B row stride (2*D cols): scatter payload is the
    # first D cols (128B descriptors at half the DMA cost of f32 rows).
    out0 = nc.dram_tensor("out0", [HOUT, 2 * D], dt.bfloat16, kind="ExternalOutput").ap()
    out1 = nc.dram_tensor("out1", [HOUT, 2 * D], dt.bfloat16, kind="ExternalOutput").ap()
    # f32 staging for tail keys (zero-donated by the runner); one tensor
    # per tail group so the groups' passes carry no false dependencies
    stages = [
        nc.dram_tensor(f"stage{g}", [max(sz, 1), 2 * D], dt.bfloat16, kind="ExternalOutput").ap()
        for g, sz in enumerate(plan["STG_G"])
    ]
    outs = (out0, out1)

    def tail_pass(tp, g2pool):
        G2 = g2pool.tile([128, MAXB, 2 * D], dt.bfloat16, tag="g2", name="g2")
        nb = tp["b_sz"] // 128
        nc.gpsimd.dma_gather(
            out_ap=G2[:, :nb, :],
            in_ap=stages[tp["grp"]][:, :],
            idxs_ap=gi2_sb[:, tp["b_lo"] // 16 : (tp["b_lo"] + tp["b_sz"]) // 16],
            num_idxs=tp["b_sz"],
            num_idxs_reg=tp["b_sz"],
            elem_size=2 * D,
            queue_num=0,
            single_packet=False,
        )
        Gb2 = g2pool.tile([128, MAXB, D], dt.bfloat16, tag="gb2", name="gb2")
        # interleave halves so the two per-half WAW chains advance together
        bl = sorted(tp["bundles"], key=lambda b: (b[1] - (10**9 if b[0] else 0)))
        bl = [b for pair in zip(bl[: len(bl) // 2 + 1], bl[len(bl) // 2 :]) for b in pair]
        seen = set()
        bl = [b for b in bl if not (b in seen or seen.add(b))]
        assert sorted(bl) == sorted(tp["bundles"]), "bundle interleave dropped entries"
        for h, boff, bsz in bl:
            c0 = (boff - tp["b_lo"]) // 128
            cn = (bsz + 127) // 128
            # compact this bundle's columns (drop the staging pad cols) so
            # its scatter fires early
            nc.vector.tensor_scalar_mul(
                out=Gb2[:, c0 : c0 + cn, :],
                in0=G2[:, c0 : c0 + cn, 0:D],
                scalar1=1.0,
            )
            nc.gpsimd.dma_scatter_add(
                out_ap=outs[h][:, :D],
                in_ap=Gb2[:, c0 : c0 + (bsz + 127) // 128, :],
                idxs_ap=si2_sb[:, boff // 16 : boff // 16 + (bsz + 15) // 16],
                num_idxs=bsz,
                num_idxs_reg=bsz,
                elem_size=D,
                elem_step=2 * D,
                queue_num=0,
                single_packet=False,
            )

    with tile.TileContext(nc) as tc:
        with (
            tc.tile_pool(name="const", bufs=1) as cpool,
            tc.tile_pool(name="g", bufs=2) as gpool,
            tc.tile_pool(name="gb", bufs=3) as bpool,
            tc.tile_pool(name="gt", bufs=2) as tpool,
        ):
            gi_sb = cpool.tile([128, TOT // 16], dt.int16, tag="gi")
            si_sb = cpool.tile([128, TOT // 16], dt.int16, tag="si")
            rc_sb = cpool.tile([128, TOT // 128], dt.float32, tag="rc")
            gi2_sb = cpool.tile([128, TOTB // 16], dt.int16, tag="gi2")
            si2_sb = cpool.tile([128, TOTB // 16], dt.int16, tag="si2")
            # chunked const loads: the first gather only waits for its
            # own slice of the index data, not the whole 8.5MB
            NCH = 4
            cw = ((TOT // 16) + NCH - 1) // NCH
            cw -= cw % 8  # keep 16B-col alignment
            for ci in range(NCH):
                lo = ci * cw
                hi = (TOT // 16) if ci == NCH - 1 else (ci + 1) * cw
                if hi <= lo:
                    continue
                nc.sync.dma_start(gi_sb[:, lo:hi], gidx[:, lo:hi])
                nc.sync.dma_start(si_sb[:, lo:hi], sidx[:, lo:hi])
                nc.sync.dma_start(rc_sb[:, lo // 8 : hi // 8], recip[:, lo // 8 : hi // 8])
            nc.sync.dma_start(gi2_sb[:], gidx2[:])
            nc.sync.dma_start(si2_sb[:], sidx2[:])

            tail_passes = {tp["after_seg"]: tp for tp in plan["tailpass"]}
            for seg_i, seg_entry in enumerate(plan["segs"]):
                s = seg_entry["seg"]
                off = seg_entry["off"]
                sz = seg_entry["gn"]  # exact gather count (pads beyond unused)
                nchunk = (sz + 127) // 128
                seg_rows = min(SEG, V - s * SEG)
                G = gpool.tile([128, MAXC, D], dt.float32, tag="g", name="g")
                nc.gpsimd.dma_gather(
                    out_ap=G[:, :nchunk, :],
                    in_ap=table[s * SEG : s * SEG + seg_rows, :],
                    idxs_ap=gi_sb[:, off // 16 : (off + sz) // 16],
                    num_idxs=sz,
                    num_idxs_reg=sz,
                    elem_size=D,
                    queue_num=0,
                    single_packet=False,
                )
                # prescale only the L0 region into Gb (the tail region is
                # separately prescaled into Gt for staging)
                t = seg_entry.get("tail")
                l0chunk = (t[0] - off) // 128 if t is not None else nchunk
                Gb = bpool.tile([128, MAXC, D], dt.bfloat16, tag="gb", name="gb")
                nc.vector.tensor_tensor(
                    out=Gb[:, :l0chunk, :],
                    in0=G[:, :l0chunk, :],
                    in1=rc_sb[:, off // 128 : off // 128 + l0chunk].to_broadcast(
                        [128, l0chunk, D]
                    ),
                    op=mybir.AluOpType.mult,
                )
                for h, loff, lsz in seg_entry["l0"]:
                    c0 = (loff - off) // 128
                    nc.gpsimd.dma_scatter_add(
                        out_ap=outs[h][:, :D],
                        in_ap=Gb[:, c0 : c0 + (lsz + 127) // 128, :],
                        idxs_ap=si_sb[:, loff // 16 : (loff + lsz) // 16],
                        num_idxs=lsz,
                        num_idxs_reg=lsz,
                        elem_size=D,
                        elem_step=2 * D,
                        queue_num=0,
                        single_packet=False,
                    )
                t = seg_entry.get("tail")
                if t is not None:
                    toff, tsz = t
                    c0 = (toff - off) // 128
                    tc_ = (tsz + 127) // 128
                    # prescaled f32 copy of the tail region for staging
                    Gt = tpool.tile([128, MAXT, D], dt.bfloat16, tag="gt", name="gt")
                    nc.vector.tensor_tensor(
                        out=Gt[:, :tc_, :],
                        in0=G[:, c0 : c0 + tc_, :],
                        in1=rc_sb[:, toff // 128 : toff // 128 + tc_].to_broadcast(
                            [128, tc_, D]
                        ),
                        op=mybir.AluOpType.mult,
                    )
                    nc.gpsimd.dma_scatter_add(
                        out_ap=stages[seg_entry["grp"]][:, :D],
                        in_ap=Gt[:, :tc_, :],
                        idxs_ap=si_sb[:, toff // 16 : toff // 16 + (tsz + 15) // 16],
                        num_idxs=tsz,
                        num_idxs_reg=tsz,
                        elem_size=D,
                        elem_step=2 * D,
                        queue_num=0,
                        single_packet=False,
                    )
                tp = tail_passes.get(seg_i)
                if tp is not None and tp["b_sz"] > 0:
                    tail_pass(tp, tpool)

    nc.compile()
    return nc


def _wrap16(a, ranges, width):
    """16-wrap the listed [off, off+sz) ranges of per-core array a into
    [CORES, 128, width] (idx j of a range -> partition j%16, col j//16,
    replicated across the 8 gpsimd sub-cores)."""
    outp = np.zeros((CORES, 128, width), a.dtype)
    for off, rsz in ranges:
        r16 = a[:, off : off + rsz].reshape(CORES, rsz // 16, 16)
        outp[:, :, off // 16 : (off + rsz) // 16] = np.tile(
            r16.transpose(0, 2, 1), (1, 8, 1)
        )
    return outp


def _assign_rows(rows, s, ROWS):
    """Deal rows into the 16 (core, half) bins so that rows with similar
    (count, segment-signature) profiles spread evenly -- equalizing the
    per-(core, seg, half) slice counts that set the padded sizes."""
    RPC = ROWS // CORES
    HR = RPC // 2
    cnt_r = np.bincount(rows, minlength=ROWS)
    ok = np.lexsort((s, rows))
    r_sorted = rows[ok]
    s_sorted = s[ok]
    first = np.concatenate([[True], r_sorted[1:] != r_sorted[:-1]])
    fidx = np.flatnonzero(first)
    pos = np.arange(len(ok)) - fidx[np.cumsum(first) - 1]
    sig = np.full((ROWS, 4), 255, np.uint8)
    m = pos < 4
    sig[r_sorted[m], pos[m]] = s_sorted[m].astype(np.uint8)
    o = np.lexsort((sig[:, 3], sig[:, 2], sig[:, 1], sig[:, 0], cnt_r))
    binof = np.empty(ROWS, np.int64)
    binof[o] = np.arange(ROWS) % (2 * CORES)
    posof = np.empty(ROWS, np.int64)
    posof[o] = np.arange(ROWS) // (2 * CORES)
    A_core = binof // 2
    A_half = binof % 2
    A_lr = A_half * HR + posof
    gmap = A_core * RPC + A_lr  # global concat index of each row
    return A_core, A_half, A_lr, gmap


def _pack(vals, rows, V, ROWS, cnt):
    RPC = ROWS // CORES
    HR = RPC // 2
    NSEG = -(-V // SEG)
    GSEGS = -(-NSEG // NGRP)  # segments per tail group
    s = vals // SEG
    A_core, A_half, A_lr, gmap = _assign_rows(rows, s, ROWS)
    core = A_core[rows]
    lr = A_lr[rows]
    h = A_half[rows]
    grp = s // GSEGS

    # lev = rank of key within (core, seg, half, row)
    key1 = ((core * 2 + h) * NSEG + s) * np.int64(ROWS) + lr
    o1 = np.argsort(key1, kind="stable")
    ks = key1[o1]
    newrun = np.concatenate([[True], ks[1:] != ks[:-1]])
    runstart = np.flatnonzero(newrun)
    lev_sorted = np.arange(len(ks)) - runstart[np.cumsum(newrun) - 1]
    lev = np.empty(len(ks), dtype=np.int64)
    lev[o1] = lev_sorted
    tail = lev >= 1

    # glev = rank of tail key within (core, group, half, row) (tail only)
    key2 = ((core * NGRP + grp) * 2 + h) * np.int64(ROWS) + lr
    o2 = np.argsort(np.where(tail, key2, np.int64(-1)), kind="stable")
    k2s = key2[o2]
    t2s = tail[o2]
    first_tail = int(np.argmax(t2s)) if t2s.any() else len(o2)
    glev = np.zeros(len(vals), dtype=np.int64)
    if first_tail < len(o2):
        sub = k2s[first_tail:]
        nr = np.concatenate([[True], sub[1:] != sub[:-1]])
        rstart = np.flatnonzero(nr)
        gl_sorted = np.arange(len(sub)) - rstart[np.cumsum(nr) - 1]
        glev[o2[first_tail:]] = gl_sorted
    NB = int(glev[tail].max()) + 1 if tail.any() else 1

    # ---- main slot space: per seg [L0h0 | L0h1 | tailslice] ----
    region = np.where(tail, 2, h)
    rid = (s * 3 + region).astype(np.int64)
    NR = NSEG * 3
    bid = core * NR + rid
    bc = np.bincount(bid, minlength=CORES * NR).reshape(CORES, NR)
    mx = bc.max(axis=0)
    n16r = ((mx + 15) // 16 * 16).astype(np.int64)  # exact descriptor counts
    szr = ((mx + 127) // 128 * 128).astype(np.int64)
    starts = np.zeros(NR + 1, dtype=np.int64)
    np.cumsum(szr, out=starts[1:])
    TOT = int(starts[-1])

    # order keys: (core, seg, region, row) -- row-sorted within each slice
    okey = bid * np.int64(ROWS) + lr
    order = np.argsort(okey, kind="stable")
    vs, ls, hs, ss, cs, bs = (
        vals[order], lr[order], h[order], s[order], core[order], bid[order],
    )
    pos_in_b = np.arange(len(order)) - np.concatenate(
        [[0], np.cumsum(np.bincount(bs, minlength=CORES * NR))]
    )[bs]
    slot = starts[bs % NR] + pos_in_b

    gi = np.zeros((CORES, TOT), np.int16)
    si = np.zeros((CORES, TOT), np.int16)
    rc = np.zeros((CORES, TOT), np.float32)
    gi[cs, slot] = (vs - ss * SEG).astype(np.int16)

    # staging layout mirrors the tail slices 1:1 (group-contiguous since
    # segments are group-ordered)
    stg_starts = np.zeros(NSEG + 1, dtype=np.int64)
    np.cumsum(n16r[np.arange(NSEG) * 3 + 2], out=stg_starts[1:])
    STG = int(stg_starts[-1])
    assert STG <= 32767, STG

    stg_base_seg = np.array(
        [int(stg_starts[(sg // GSEGS) * GSEGS]) for sg in range(NSEG)],
        dtype=np.int64,
    )
    for b in range(NR):
        sg, rg = divmod(b, 3)
        lo, hi_ = int(starts[b]), int(starts[b + 1])
        if rg == 2:
            # group-relative staging slot (out_ap is the group's slice)
            si[:, lo:hi_] = (
                stg_starts[sg] - stg_base_seg[sg] + np.arange(hi_ - lo)
            ).astype(np.int16)
        else:
            si[:, lo:hi_] = (HR + (np.arange(hi_ - lo) % HALF_PAD)).astype(np.int16)
    m0 = ~tail[order]
    si[cs[m0], slot[m0]] = (ls[m0] - hs[m0] * HR).astype(np.int16)
    rc[cs, slot] = (1.0 / np.maximum(cnt, 1.0))[rows[order]].astype(np.float32)

    # staging slot per tail key
    staged = tail
    stg_slot = np.full(len(vals), -1, dtype=np.int64)
    tmask = staged[order]
    stg_slot[order[tmask]] = (
        stg_starts[ss[tmask]] + (slot[tmask] - starts[bs[tmask] % NR])
    )

    # ---- bundle slot space: per (group, half, glev) ----
    NBT = NGRP * 2 * NB
    bnd = ((grp * 2 + h) * NB + glev).astype(np.int64)
    bndid = np.where(staged, core * NBT + bnd, -1)
    bcnt = np.bincount(bndid[staged], minlength=CORES * NBT).reshape(CORES, NBT)
    bmx = bcnt.max(axis=0)
    b16 = ((bmx + 15) // 16 * 16).astype(np.int64)
    szb = ((bmx + 127) // 128 * 128).astype(np.int64)
    bstarts = np.zeros(NBT + 1, dtype=np.int64)
    np.cumsum(szb, out=bstarts[1:])
    TOTB = max(int(bstarts[-1]), 128)
    if TOTB % 128:
        TOTB = (TOTB + 127) // 128 * 128

    ot = np.flatnonzero(staged)
    okey2 = bndid[ot] * np.int64(ROWS) + lr[ot]
    o3 = ot[np.argsort(okey2, kind="stable")]
    bs2 = bndid[o3]
    pos2 = np.arange(len(o3)) - np.concatenate(
        [[0], np.cumsum(np.bincount(bs2, minlength=CORES * NBT))]
    )[bs2]
    slot2 = bstarts[bs2 % NBT] + pos2

    gi2 = np.zeros((CORES, TOTB), np.int16)
    si2 = np.zeros((CORES, TOTB), np.int16)
    for b in range(NBT):
        lo, hi_ = int(bstarts[b]), int(bstarts[b + 1])
        si2[:, lo:hi_] = (HR + (np.arange(hi_ - lo) % HALF_PAD)).astype(np.int16)
    c3 = core[o3]
    # gather idx relative to the group's staging base
    g3 = grp[o3] if hasattr(grp, "__getitem__") else grp
    stg_base_of_grp = np.array(
        [int(stg_starts[min(g * GSEGS, NSEG)]) for g in range(NGRP + 1)],
        dtype=np.int64,
    )
    gi2[c3, slot2] = (stg_slot[o3] - stg_base_of_grp[grp[o3]]).astype(np.int16)
    si2[c3, slot2] = (lr[o3] - h[o3] * HR).astype(np.int16)

    # ---- wrap + plan ----
    seg_ranges = []
    plan_segs = []
    for sg in range(NSEG):
        off = int(starts[sg * 3])
        end = int(starts[sg * 3 + 3])
        if end == off:
            continue
        tb = sg * 3 + 2
        l0 = []
        last_direct = None
        for rg in range(2):
            b = sg * 3 + rg
            if szr[b] > 0:
                l0.append((rg, int(starts[b]), int(n16r[b])))
                last_direct = b
        e = {"seg": sg, "off": off, "l0": l0}
        if szr[tb] > 0:
            e["tail"] = (int(starts[tb]), int(n16r[tb]))
            e["grp"] = sg // GSEGS
            e["gn"] = int(starts[tb] + n16r[tb] - off)
        else:
            e["gn"] = int(starts[last_direct] + n16r[last_direct] - off)
        plan_segs.append(e)
        seg_ranges.append((off, end - off))

    # tail passes: after the last entry that writes the group's staging
    tailpass = []
    for g in range(NGRP):
        sg_lo, sg_hi = g * GSEGS, min((g + 1) * GSEGS, NSEG)
        after = max(
            (
                i
                for i, e in enumerate(plan_segs)
                if "tail" in e and sg_lo <= e["seg"] < sg_hi
            ),
            default=None,
        )
        if after is None:
            continue
        b_lo = int(bstarts[g * 2 * NB])
        b_hi = int(bstarts[(g + 1) * 2 * NB])
        bundles = []
        for b in range(g * 2 * NB, (g + 1) * 2 * NB):
            if szb[b] > 0:
                hh = (b // NB) % 2
                bundles.append((hh, int(bstarts[b]), int(b16[b])))
        tailpass.append(
            {
                "after_seg": after,
                "grp": g,
                "b_lo": b_lo,
                "b_sz": b_hi - b_lo,
                "bundles": bundles,
            }
        )

    si_ranges = [(int(starts[b]), int(szr[b])) for b in range(NR) if szr[b] > 0]
    gi_w = _wrap16(gi, seg_ranges, TOT // 16)
    si_w = _wrap16(si, si_ranges, TOT // 16)
    gi2_ranges = [(tp["b_lo"], tp["b_sz"]) for tp in tailpass if tp["b_sz"] > 0]
    gi2_w = _wrap16(gi2, gi2_ranges, TOTB // 16)
    si2_w = _wrap16(
        si2,
        [(int(bstarts[b]), int(szb[b])) for b in range(NBT) if szb[b] > 0],
        TOTB // 16,
    )
    rc_dev = np.zeros((CORES, 128, TOT // 128), np.float32)
    for off, rsz in seg_ranges:
        segm = rc[:, off : off + rsz].reshape(CORES, rsz // 128, 128)
        rc_dev[:, :, off // 128 : (off + rsz) // 128] = segm.transpose(0, 2, 1)

    plan = {
        "TOT": TOT,
        "STG": max(STG, 128),
        "STG_G": [
            int(stg_base_of_grp[g + 1]) - int(stg_base_of_grp[g])
            for g in range(NGRP)
        ],
        "TOTB": TOTB,
        "MAXC": max((e["gn"] + 127) // 128 for e in plan_segs),
        "MAXT": (max((e["tail"][1] for e in plan_segs if "tail" in e), default=128) + 127)
        // 128,
        "MAXB": (max((tp["b_sz"] for tp in tailpass), default=128) + 127) // 128,
        "segs": plan_segs,
        "tailpass": tailpass,
    }
    plan["gmap"] = gmap
    return gi_w, si_w, rc_dev, gi2_w, si2_w, plan, HR


def kernel(table, values, row_indices):
    from concourse.bass_utils import run_bass_kernel_spmd

    table = np.ascontiguousarray(np.asarray(table), dtype=np.float32)
    vals = np.asarray(values).astype(np.int64)
    rows = np.asarray(row_indices).astype(np.int64)
    V, D = table.shape
    B, S = 16384, 26
    ROWS = B * S
    cnt = np.bincount(rows, minlength=ROWS).astype(np.float32)

    gi_w, si_w, rc_dev, gi2_w, si2_w, plan, HR = _pack(vals, rows, V, ROWS, cnt)
    nc = _build_program(V, D, HR, plan)
    global _last_nc
    _last_nc = nc

    in_maps = [
        {
            "table": table,
            "gidx": np.ascontiguousarray(gi_w[c]),
            "sidx": np.ascontiguousarray(si_w[c]),
            "recip": np.ascontiguousarray(rc_dev[c]),
            "gidx2": np.ascontiguousarray(gi2_w[c]),
            "sidx2": np.ascontiguousarray(si2_w[c]),
        }
        for c in range(CORES)
    ]
    res = run_bass_kernel_spmd(nc, in_maps, core_ids=list(range(CORES)))
    global _last_results
    _last_results = res
    outs = [
        np.concatenate(
            [
                np.asarray(res.results[c]["out0"])[:HR, :D].astype(np.float32),
                np.asarray(res.results[c]["out1"])[:HR, :D].astype(np.float32),
            ],
            axis=0,
        )
        for c in range(CORES)
    ]
    full = np.concatenate(outs, axis=0)[plan["gmap"]]
    return np.ascontiguousarray(full.reshape(B, S, D), dtype=np.float32)



# revision 8
# speedup vs baseline: 1.4346x; 1.4346x over previous
"""Distributed embedding lookup v7: bf16 table with 128-byte gather
descriptors + bf16 dma_scatter_add with a staging pass for duplicate-row
keys.

v7 over v6: the table is uploaded as bf16 padded to 256B rows ([V, 128]
bf16), and all gathers fetch 64-element (128B) payloads on a 256B row
stride via a raw InstDMAGatherAnt (the bass wrapper's elem%256B assert
is an over-broad transpose restriction; the descriptor format only
needs the *stride* in 256B units, verified exact on HW).  In the cost
model a sub-512B descriptor costs max(bytes*2/22.5, 7) ns, so halving
the payload from 256B to 128B halves the dominant gather cost.

Sharding: output rows split contiguously over 8 cores (53,248 rows each).
Keys are routed host-side into per-core buckets keyed by vocab segment
(32,768 rows, int16 gather indices).  The HW CCE scatter-add loses
updates when one instruction carries two descriptors targeting the same
output row, so every scatter instruction must be row-unique:

- level-0 keys (rank 0 within (core, seg, half, row), ~97%) scatter
  directly per (segment, half) -- row-unique by construction; Tile's
  WAW chain serializes across instructions.
- tail keys (rank >= 1) are prescaled to f32 and scattered into a
  compact staging buffer at per-key unique slots (collision-free), then
  re-gathered in (half, global-level, row) order and added to the
  output by bundle scatters (one per (half, global tail level), each
  row-unique).  The tail pass runs per segment-GROUP (two groups) so
  the first group's re-gather and bundle scatters overlap the second
  group's main sweep instead of serializing at the end.

The mean combine is a per-key 1/count pre-scale fused with the dtype
convert, so output scatters move 128B bf16 descriptors into a
256B-stride bf16 output at half the f32 DMA cost.  Pad slots use gather
index 0 with recip=0 and scatter into per-half dummy rows (or their own
unique staging slot), keeping every index count compile-time static so
one NEFF serves all 8 cores.
"""

import numpy as np

CORES = 8
SEG = 32768  # vocab rows per gather segment (int16 index range)
HALF_PAD = 128  # dummy scatter rows appended per output half
NGRP = 1  # tail-pass segment groups


def _dma_gather_narrow(gp, out_ap, in_ap, idxs_ap, num_idxs, elem_size, elem_step):
    """gpsimd.dma_gather minus the elem_size_bytes % 256 assert: gathers
    elem_size-element payloads from rows strided elem_step elements apart
    (elem_step * dtype_size must be a multiple of 256 bytes)."""
    import concourse.ap_utils as ap_utils
    import concourse.mybir as mybir

    assert idxs_ap.dtype == mybir.dt.int16
    assert in_ap.dtype == out_ap.dtype
    assert ap_utils.ap_is_contiguous(in_ap.ap[1:])
    assert ap_utils.ap_is_contiguous(out_ap.ap[1:])
    assert ap_utils.ap_is_contiguous(idxs_ap.ap[1:])
    assert in_ap.ap[-1][1] == out_ap.ap[-1][1] == elem_size
    assert in_ap.ap[0][0] == elem_step
    stride_bytes = elem_step * mybir.dt.size(in_ap.dtype)
    assert stride_bytes % 256 == 0 and stride_bytes // 256 < 256
    _in_ap = gp.lower_ap_dma(in_ap, for_custom_bir_dma=True)
    _idxs_ap = gp.lower_ap(idxs_ap)
    _out_ap = gp.lower_ap(out_ap)
    return gp.add_instruction(
        mybir.InstDMAGatherAnt(
            name=gp.bass.get_next_instruction_name(),
            ins=[*_in_ap, _idxs_ap, gp.lower_val_access(gp.to_reg(num_idxs))],
            outs=[_out_ap],
            transpose=False,
            num_idxs=num_idxs,
            elem_size=elem_size,
            stride_bytes_256=stride_bytes // 256,
            gen_mode=0,
            single_packet=False,
            queue_num=0,
            sbuf_tokens_per_rank=0,
            sbuf_free_dim_per_rank=0,
            sbuf_free_dim_pad_per_rank=0,
            sbuf_byte_offset=0,
        )
    )


def _build_program(V, D, HR, plan):
    import concourse.bacc as bacc
    import concourse.mybir as mybir
    import concourse.tile as tile
    from concourse.ap import AP as APc

    dt = mybir.dt
    TOT = plan["TOT"]
    STG = plan["STG"]
    TOTB = plan["TOTB"]
    MAXC = plan["MAXC"]
    MAXT = plan["MAXT"]
    MAXB = plan["MAXB"]
    HOUT = HR + HALF_PAD

    nc = bacc.Bacc("TRN2", target_bir_lowering=False, debug=False)
    # bf16 table padded to 256B rows: gather descs carry 128B payloads on a
    # 256B stride
    table = nc.dram_tensor("table", [V, 2 * D], dt.bfloat16, kind="ExternalInput").ap()
    gidx = nc.dram_tensor("gidx", [128, TOT // 16], dt.int16, kind="ExternalInput").ap()
    sidx = nc.dram_tensor("sidx", [128, TOT // 16], dt.int16, kind="ExternalInput").ap()
    recip = nc.dram_tensor("recip", [128, TOT // 128], dt.float32, kind="ExternalInput").ap()
    gidx2 = nc.dram_tensor("gidx2", [128, TOTB // 16], dt.int16, kind="ExternalInput").ap()
    sidx2 = nc.dram_tensor("sidx2", [128, TOTB // 16], dt.int16, kind="ExternalInput").ap()
    # bf16 outputs with 256B row stride (2*D cols): scatter payload is the
    # first D cols (128B descriptors at half the DMA cost of f32 rows).
    out0 = nc.dram_tensor("out0", [HOUT, 2 * D], dt.bfloat16, kind="ExternalOutput").ap()
    out1 = nc.dram_tensor("out1", [HOUT, 2 * D], dt.bfloat16, kind="ExternalOutput").ap()
    # f32 staging for tail keys (zero-donated by the runner); one tensor
    # per tail group so the groups' passes carry no false dependencies
    stages = [
        nc.dram_tensor(f"stage{g}", [max(sz, 1), 2 * D], dt.bfloat16, kind="ExternalOutput").ap()
        for g, sz in enumerate(plan["STG_G"])
    ]
    outs = (out0, out1)

    def tail_pass(tp, g2pool):
        G2 = g2pool.tile([128, MAXB, D], dt.bfloat16, tag="g2", name="g2")
        nb = tp["b_sz"] // 128
        stg = stages[tp["grp"]]
        stg_rows = stg.shape[0]
        _dma_gather_narrow(
            nc.gpsimd,
            G2[:, :nb, :],
            APc(stg.tensor, 0, [[2 * D, stg_rows], [1, D]]),
            gi2_sb[:, tp["b_lo"] // 16 : (tp["b_lo"] + tp["b_sz"]) // 16],
            tp["b_sz"],
            D,
            2 * D,
        )
        # interleave halves so the two per-half WAW chains advance together
        bl = sorted(tp["bundles"], key=lambda b: (b[1] - (10**9 if b[0] else 0)))
        bl = [b for pair in zip(bl[: len(bl) // 2 + 1], bl[len(bl) // 2 :]) for b in pair]
        seen = set()
        bl = [b for b in bl if not (b in seen or seen.add(b))]
        assert sorted(bl) == sorted(tp["bundles"]), "bundle interleave dropped entries"
        for h, boff, bsz in bl:
            c0 = (boff - tp["b_lo"]) // 128
            nc.gpsimd.dma_scatter_add(
                out_ap=outs[h][:, :D],
                in_ap=G2[:, c0 : c0 + (bsz + 127) // 128, :],
                idxs_ap=si2_sb[:, boff // 16 : boff // 16 + (bsz + 15) // 16],
                num_idxs=bsz,
                num_idxs_reg=bsz,
                elem_size=D,
                elem_step=2 * D,
                queue_num=0,
                single_packet=False,
            )

    with tile.TileContext(nc) as tc:
        with (
            tc.tile_pool(name="const", bufs=1) as cpool,
            tc.tile_pool(name="g", bufs=2) as gpool,
            tc.tile_pool(name="gb", bufs=3) as bpool,
            tc.tile_pool(name="gt", bufs=2) as tpool,
        ):
            gi_sb = cpool.tile([128, TOT // 16], dt.int16, tag="gi")
            si_sb = cpool.tile([128, TOT // 16], dt.int16, tag="si")
            rc_sb = cpool.tile([128, TOT // 128], dt.float32, tag="rc")
            gi2_sb = cpool.tile([128, TOTB // 16], dt.int16, tag="gi2")
            si2_sb = cpool.tile([128, TOTB // 16], dt.int16, tag="si2")
            # chunked const loads: the first gather only waits for its
            # own slice of the index data, not the whole 8.5MB
            NCH = 4
            cw = ((TOT // 16) + NCH - 1) // NCH
            cw -= cw % 8  # keep 16B-col alignment
            for ci in range(NCH):
                lo = ci * cw
                hi = (TOT // 16) if ci == NCH - 1 else (ci + 1) * cw
                if hi <= lo:
                    continue
                nc.sync.dma_start(gi_sb[:, lo:hi], gidx[:, lo:hi])
                nc.sync.dma_start(si_sb[:, lo:hi], sidx[:, lo:hi])
                nc.sync.dma_start(rc_sb[:, lo // 8 : hi // 8], recip[:, lo // 8 : hi // 8])
            nc.sync.dma_start(gi2_sb[:], gidx2[:])
            nc.sync.dma_start(si2_sb[:], sidx2[:])

            tail_passes = {tp["after_seg"]: tp for tp in plan["tailpass"]}
            for seg_i, seg_entry in enumerate(plan["segs"]):
                s = seg_entry["seg"]
                off = seg_entry["off"]
                sz = seg_entry["gn"]  # exact gather count (pads beyond unused)
                nchunk = (sz + 127) // 128
                seg_rows = min(SEG, V - s * SEG)
                G = gpool.tile([128, MAXC, D], dt.bfloat16, tag="g", name="g")
                _dma_gather_narrow(
                    nc.gpsimd,
                    G[:, :nchunk, :],
                    APc(table.tensor, s * SEG * 2 * D, [[2 * D, seg_rows], [1, D]]),
                    gi_sb[:, off // 16 : (off + sz) // 16],
                    sz,
                    D,
                    2 * D,
                )
                # prescale only the L0 region into Gb (the tail region is
                # separately prescaled into Gt for staging)
                t = seg_entry.get("tail")
                l0chunk = (t[0] - off) // 128 if t is not None else nchunk
                Gb = bpool.tile([128, MAXC, D], dt.bfloat16, tag="gb", name="gb")
                nc.vector.tensor_tensor(
                    out=Gb[:, :l0chunk, :],
                    in0=G[:, :l0chunk, :],
                    in1=rc_sb[:, off // 128 : off // 128 + l0chunk].to_broadcast(
                        [128, l0chunk, D]
                    ),
                    op=mybir.AluOpType.mult,
                )
                for h, loff, lsz in seg_entry["l0"]:
                    c0 = (loff - off) // 128
                    nc.gpsimd.dma_scatter_add(
                        out_ap=outs[h][:, :D],
                        in_ap=Gb[:, c0 : c0 + (lsz + 127) // 128, :],
                        idxs_ap=si_sb[:, loff // 16 : (loff + lsz) // 16],
                        num_idxs=lsz,
                        num_idxs_reg=lsz,
                        elem_size=D,
                        elem_step=2 * D,
                        queue_num=0,
                        single_packet=False,
                    )
                t = seg_entry.get("tail")
                if t is not None:
                    toff, tsz = t
                    c0 = (toff - off) // 128
                    tc_ = (tsz + 127) // 128
                    # prescaled f32 copy of the tail region for staging
                    Gt = tpool.tile([128, MAXT, D], dt.bfloat16, tag="gt", name="gt")
                    nc.vector.tensor_tensor(
                        out=Gt[:, :tc_, :],
                        in0=G[:, c0 : c0 + tc_, :],
                        in1=rc_sb[:, toff // 128 : toff // 128 + tc_].to_broadcast(
                            [128, tc_, D]
                        ),
                        op=mybir.AluOpType.mult,
                    )
                    nc.gpsimd.dma_scatter_add(
                        out_ap=stages[seg_entry["grp"]][:, :D],
                        in_ap=Gt[:, :tc_, :],
                        idxs_ap=si_sb[:, toff // 16 : toff // 16 + (tsz + 15) // 16],
                        num_idxs=tsz,
                        num_idxs_reg=tsz,
                        elem_size=D,
                        elem_step=2 * D,
                        queue_num=0,
                        single_packet=False,
                    )
                tp = tail_passes.get(seg_i)
                if tp is not None and tp["b_sz"] > 0:
                    tail_pass(tp, tpool)

    nc.compile()
    return nc


def _wrap16(a, ranges, width):
    """16-wrap the listed [off, off+sz) ranges of per-core array a into
    [CORES, 128, width] (idx j of a range -> partition j%16, col j//16,
    replicated across the 8 gpsimd sub-cores)."""
    outp = np.zeros((CORES, 128, width), a.dtype)
    for off, rsz in ranges:
        r16 = a[:, off : off + rsz].reshape(CORES, rsz // 16, 16)
        outp[:, :, off // 16 : (off + rsz) // 16] = np.tile(
            r16.transpose(0, 2, 1), (1, 8, 1)
        )
    return outp


def _assign_rows(rows, s, ROWS):
    """Deal rows into the 16 (core, half) bins so that rows with similar
    (count, segment-signature) profiles spread evenly -- equalizing the
    per-(core, seg, half) slice counts that set the padded sizes."""
    RPC = ROWS // CORES
    HR = RPC // 2
    cnt_r = np.bincount(rows, minlength=ROWS)
    ok = np.lexsort((s, rows))
    r_sorted = rows[ok]
    s_sorted = s[ok]
    first = np.concatenate([[True], r_sorted[1:] != r_sorted[:-1]])
    fidx = np.flatnonzero(first)
    pos = np.arange(len(ok)) - fidx[np.cumsum(first) - 1]
    sig = np.full((ROWS, 4), 255, np.uint8)
    m = pos < 4
    sig[r_sorted[m], pos[m]] = s_sorted[m].astype(np.uint8)
    o = np.lexsort((sig[:, 3], sig[:, 2], sig[:, 1], sig[:, 0], cnt_r))
    binof = np.empty(ROWS, np.int64)
    binof[o] = np.arange(ROWS) % (2 * CORES)
    posof = np.empty(ROWS, np.int64)
    posof[o] = np.arange(ROWS) // (2 * CORES)
    A_core = binof // 2
    A_half = binof % 2
    A_lr = A_half * HR + posof
    gmap = A_core * RPC + A_lr  # global concat index of each row
    return A_core, A_half, A_lr, gmap


def _pack(vals, rows, V, ROWS, cnt):
    RPC = ROWS // CORES
    HR = RPC // 2
    NSEG = -(-V // SEG)
    GSEGS = -(-NSEG // NGRP)  # segments per tail group
    s = vals // SEG
    A_core, A_half, A_lr, gmap = _assign_rows(rows, s, ROWS)
    core = A_core[rows]
    lr = A_lr[rows]
    h = A_half[rows]
    grp = s // GSEGS

    # lev = rank of key within (core, seg, half, row)
    key1 = ((core * 2 + h) * NSEG + s) * np.int64(ROWS) + lr
    o1 = np.argsort(key1, kind="stable")
    ks = key1[o1]
    newrun = np.concatenate([[True], ks[1:] != ks[:-1]])
    runstart = np.flatnonzero(newrun)
    lev_sorted = np.arange(len(ks)) - runstart[np.cumsum(newrun) - 1]
    lev = np.empty(len(ks), dtype=np.int64)
    lev[o1] = lev_sorted
    tail = lev >= 1

    # glev = rank of tail key within (core, group, half, row) (tail only)
    key2 = ((core * NGRP + grp) * 2 + h) * np.int64(ROWS) + lr
    o2 = np.argsort(np.where(tail, key2, np.int64(-1)), kind="stable")
    k2s = key2[o2]
    t2s = tail[o2]
    first_tail = int(np.argmax(t2s)) if t2s.any() else len(o2)
    glev = np.zeros(len(vals), dtype=np.int64)
    if first_tail < len(o2):
        sub = k2s[first_tail:]
        nr = np.concatenate([[True], sub[1:] != sub[:-1]])
        rstart = np.flatnonzero(nr)
        gl_sorted = np.arange(len(sub)) - rstart[np.cumsum(nr) - 1]
        glev[o2[first_tail:]] = gl_sorted
    NB = int(glev[tail].max()) + 1 if tail.any() else 1

    # ---- main slot space: per seg [L0h0 | L0h1 | tailslice] ----
    region = np.where(tail, 2, h)
    rid = (s * 3 + region).astype(np.int64)
    NR = NSEG * 3
    bid = core * NR + rid
    bc = np.bincount(bid, minlength=CORES * NR).reshape(CORES, NR)
    mx = bc.max(axis=0)
    n16r = ((mx + 15) // 16 * 16).astype(np.int64)  # exact descriptor counts
    szr = ((mx + 127) // 128 * 128).astype(np.int64)
    starts = np.zeros(NR + 1, dtype=np.int64)
    np.cumsum(szr, out=starts[1:])
    TOT = int(starts[-1])

    # order keys: (core, seg, region, row) -- row-sorted within each slice
    okey = bid * np.int64(ROWS) + lr
    order = np.argsort(okey, kind="stable")
    vs, ls, hs, ss, cs, bs = (
        vals[order], lr[order], h[order], s[order], core[order], bid[order],
    )
    pos_in_b = np.arange(len(order)) - np.concatenate(
        [[0], np.cumsum(np.bincount(bs, minlength=CORES * NR))]
    )[bs]
    slot = starts[bs % NR] + pos_in_b

    gi = np.zeros((CORES, TOT), np.int16)
    si = np.zeros((CORES, TOT), np.int16)
    rc = np.zeros((CORES, TOT), np.float32)
    gi[cs, slot] = (vs - ss * SEG).astype(np.int16)

    # staging layout mirrors the tail slices 1:1 (group-contiguous since
    # segments are group-ordered)
    stg_starts = np.zeros(NSEG + 1, dtype=np.int64)
    np.cumsum(n16r[np.arange(NSEG) * 3 + 2], out=stg_starts[1:])
    STG = int(stg_starts[-1])
    assert STG <= 32767, STG

    stg_base_seg = np.array(
        [int(stg_starts[(sg // GSEGS) * GSEGS]) for sg in range(NSEG)],
        dtype=np.int64,
    )
    for b in range(NR):
        sg, rg = divmod(b, 3)
        lo, hi_ = int(starts[b]), int(starts[b + 1])
        if rg == 2:
            # group-relative staging slot (out_ap is the group's slice)
            si[:, lo:hi_] = (
                stg_starts[sg] - stg_base_seg[sg] + np.arange(hi_ - lo)
            ).astype(np.int16)
        else:
            si[:, lo:hi_] = (HR + (np.arange(hi_ - lo) % HALF_PAD)).astype(np.int16)
    m0 = ~tail[order]
    si[cs[m0], slot[m0]] = (ls[m0] - hs[m0] * HR).astype(np.int16)
    rc[cs, slot] = (1.0 / np.maximum(cnt, 1.0))[rows[order]].astype(np.float32)

    # staging slot per tail key
    staged = tail
    stg_slot = np.full(len(vals), -1, dtype=np.int64)
    tmask = staged[order]
    stg_slot[order[tmask]] = (
        stg_starts[ss[tmask]] + (slot[tmask] - starts[bs[tmask] % NR])
    )

    # ---- bundle slot space: per (group, half, glev) ----
    NBT = NGRP * 2 * NB
    bnd = ((grp * 2 + h) * NB + glev).astype(np.int64)
    bndid = np.where(staged, core * NBT + bnd, -1)
    bcnt = np.bincount(bndid[staged], minlength=CORES * NBT).reshape(CORES, NBT)
    bmx = bcnt.max(axis=0)
    b16 = ((bmx + 15) // 16 * 16).astype(np.int64)
    szb = ((bmx + 127) // 128 * 128).astype(np.int64)
    bstarts = np.zeros(NBT + 1, dtype=np.int64)
    np.cumsum(szb, out=bstarts[1:])
    TOTB = max(int(bstarts[-1]), 128)
    if TOTB % 128:
        TOTB = (TOTB + 127) // 128 * 128

    ot = np.flatnonzero(staged)
    okey2 = bndid[ot] * np.int64(ROWS) + lr[ot]
    o3 = ot[np.argsort(okey2, kind="stable")]
    bs2 = bndid[o3]
    pos2 = np.arange(len(o3)) - np.concatenate(
        [[0], np.cumsum(np.bincount(bs2, minlength=CORES * NBT))]
    )[bs2]
    slot2 = bstarts[bs2 % NBT] + pos2

    gi2 = np.zeros((CORES, TOTB), np.int16)
    si2 = np.zeros((CORES, TOTB), np.int16)
    for b in range(NBT):
        lo, hi_ = int(bstarts[b]), int(bstarts[b + 1])
        si2[:, lo:hi_] = (HR + (np.arange(hi_ - lo) % HALF_PAD)).astype(np.int16)
    c3 = core[o3]
    # gather idx relative to the group's staging base
    g3 = grp[o3] if hasattr(grp, "__getitem__") else grp
    stg_base_of_grp = np.array(
        [int(stg_starts[min(g * GSEGS, NSEG)]) for g in range(NGRP + 1)],
        dtype=np.int64,
    )
    gi2[c3, slot2] = (stg_slot[o3] - stg_base_of_grp[grp[o3]]).astype(np.int16)
    si2[c3, slot2] = (lr[o3] - h[o3] * HR).astype(np.int16)

    # ---- wrap + plan ----
    seg_ranges = []
    plan_segs = []
    for sg in range(NSEG):
        off = int(starts[sg * 3])
        end = int(starts[sg * 3 + 3])
        if end == off:
            continue
        tb = sg * 3 + 2
        l0 = []
        last_direct = None
        for rg in range(2):
            b = sg * 3 + rg
            if szr[b] > 0:
                l0.append((rg, int(starts[b]), int(n16r[b])))
                last_direct = b
        e = {"seg": sg, "off": off, "l0": l0}
        if szr[tb] > 0:
            e["tail"] = (int(starts[tb]), int(n16r[tb]))
            e["grp"] = sg // GSEGS
            e["gn"] = int(starts[tb] + n16r[tb] - off)
        else:
            e["gn"] = int(starts[last_direct] + n16r[last_direct] - off)
        plan_segs.append(e)
        seg_ranges.append((off, end - off))

    # tail passes: after the last entry that writes the group's staging
    tailpass = []
    for g in range(NGRP):
        sg_lo, sg_hi = g * GSEGS, min((g + 1) * GSEGS, NSEG)
        after = max(
            (
                i
                for i, e in enumerate(plan_segs)
                if "tail" in e and sg_lo <= e["seg"] < sg_hi
            ),
            default=None,
        )
        if after is None:
            continue
        b_lo = int(bstarts[g * 2 * NB])
        b_hi = int(bstarts[(g + 1) * 2 * NB])
        bundles = []
        for b in range(g * 2 * NB, (g + 1) * 2 * NB):
            if szb[b] > 0:
                hh = (b // NB) % 2
                bundles.append((hh, int(bstarts[b]), int(b16[b])))
        tailpass.append(
            {
                "after_seg": after,
                "grp": g,
                "b_lo": b_lo,
                "b_sz": b_hi - b_lo,
                "bundles": bundles,
            }
        )

    si_ranges = [(int(starts[b]), int(szr[b])) for b in range(NR) if szr[b] > 0]
    gi_w = _wrap16(gi, seg_ranges, TOT // 16)
    si_w = _wrap16(si, si_ranges, TOT // 16)
    gi2_ranges = [(tp["b_lo"], tp["b_sz"]) for tp in tailpass if tp["b_sz"] > 0]
    gi2_w = _wrap16(gi2, gi2_ranges, TOTB // 16)
    si2_w = _wrap16(
        si2,
        [(int(bstarts[b]), int(szb[b])) for b in range(NBT) if szb[b] > 0],
        TOTB // 16,
    )
    rc_dev = np.zeros((CORES, 128, TOT // 128), np.float32)
    for off, rsz in seg_ranges:
        segm = rc[:, off : off + rsz].reshape(CORES, rsz // 128, 128)
        rc_dev[:, :, off // 128 : (off + rsz) // 128] = segm.transpose(0, 2, 1)

    plan = {
        "TOT": TOT,
        "STG": max(STG, 128),
        "STG_G": [
            int(stg_base_of_grp[g + 1]) - int(stg_base_of_grp[g])
            for g in range(NGRP)
        ],
        "TOTB": TOTB,
        "MAXC": max((e["gn"] + 127) // 128 for e in plan_segs),
        "MAXT": (max((e["tail"][1] for e in plan_segs if "tail" in e), default=128) + 127)
        // 128,
        "MAXB": (max((tp["b_sz"] for tp in tailpass), default=128) + 127) // 128,
        "segs": plan_segs,
        "tailpass": tailpass,
    }
    plan["gmap"] = gmap
    return gi_w, si_w, rc_dev, gi2_w, si2_w, plan, HR


def kernel(table, values, row_indices):
    import ml_dtypes
    from concourse.bass_utils import run_bass_kernel_spmd

    table = np.asarray(table)
    tbl16 = np.zeros((table.shape[0], 2 * table.shape[1]), dtype=ml_dtypes.bfloat16)
    tbl16[:, : table.shape[1]] = table.astype(ml_dtypes.bfloat16)
    vals = np.asarray(values).astype(np.int64)
    rows = np.asarray(row_indices).astype(np.int64)
    V, D = table.shape
    B, S = 16384, 26
    ROWS = B * S
    cnt = np.bincount(rows, minlength=ROWS).astype(np.float32)

    gi_w, si_w, rc_dev, gi2_w, si2_w, plan, HR = _pack(vals, rows, V, ROWS, cnt)
    nc = _build_program(V, D, HR, plan)
    global _last_nc
    _last_nc = nc

    in_maps = [
        {
            "table": tbl16,
            "gidx": np.ascontiguousarray(gi_w[c]),
            "sidx": np.ascontiguousarray(si_w[c]),
            "recip": np.ascontiguousarray(rc_dev[c]),
            "gidx2": np.ascontiguousarray(gi2_w[c]),
            "sidx2": np.ascontiguousarray(si2_w[c]),
        }
        for c in range(CORES)
    ]
    res = run_bass_kernel_spmd(nc, in_maps, core_ids=list(range(CORES)))
    global _last_results
    _last_results = res
    outs = [
        np.concatenate(
            [
                np.asarray(res.results[c]["out0"])[:HR, :D].astype(np.float32),
                np.asarray(res.results[c]["out1"])[:HR, :D].astype(np.float32),
            ],
            axis=0,
        )
        for c in range(CORES)
    ]
    full = np.concatenate(outs, axis=0)[plan["gmap"]]
    return np.ascontiguousarray(full.reshape(B, S, D), dtype=np.float32)

